# revision 21
# baseline (speedup 1.0000x reference)
"""Self-contained Trainium2 Bass kernel for nn_Attention (dense transformer MHA block).

Full inputs in, full outputs out. Sharding: batch (B=8) data-parallel across the
8 NeuronCores -- one batch element per core, weights replicated. No collectives.

Per-core math (x: [1024, 768], H=12 heads, D=64):
  qkv = x @ qkv_w.T ; q,k,v split ; per head: softmax(q k^T / 8) v ; proj + bias.

Layout/precision strategy:
  - All matmuls in float32r (fp32 storage, PE truncates operands, ~1.5e-4 rel
    per matmul, 4x the throughput of true fp32 on the PE). PSUM accumulation
    stays fp32. Input DRAM tensors are declared float32r directly (the PE
    rounds on read; pre-rounding producers is a formality).
  - x^T and W^T produced on-chip via PE transposes (fp32 has no DMA transpose).
  - q^T,k^T computed in [o, i] layout -> directly usable as the
    S^T = k^T.T @ q^T matmul operands (contraction over d on partitions).
  - v computed in natural [token, feature] layout with an extra ones column;
    O' = [v | 1].T @ E^T yields the attention output AND the softmax row-sums
    in one matmul (65-column trick) -- no partition-axis reduction, no
    transpose of the attention matrix anywhere.
  - softmax without max-subtraction (scores ~N(0,1); fp32 exp is safe).
  - normalization: approx reciprocal (2 ULP) of the rowsum row, broadcast over
    partitions via a DRAM round-trip DMA, one in-place multiply per head.
  - schedule: prelude (transposes + v) then an interleaved loop producing each
    head-pair's q^T/k^T right before that pair's attention, so the PE-heavy
    QKV work overlaps the ACT-heavy softmax exp work.
  - attnout lands directly in [feature, token] layout = proj's lhsT; proj
    output is natural [token, feature] and DMAs straight out.
"""

import os
import sys

for _p in ("/opt/trn_rl_repo",):
    if os.path.isdir(_p) and _p not in sys.path:
        sys.path.insert(0, _p)

import numpy as np

P = 128
N = 1024          # tokens per batch element
C = 768           # model dim
H = 12            # heads
D = 64            # head dim
B = 8             # batch (== n cores)
NB = N // P       # 8 token blocks
CB = C // P       # 6 feature blocks
SCALE = D ** -0.5  # 0.125


def build_attention_bass():
    import concourse.mybir as mybir
    import concourse.tile as tile
    from concourse import bacc
    from concourse.masks import make_identity

    f32 = mybir.dt.float32
    f32r = mybir.dt.float32r
    nc = bacc.Bacc("TRN2", target_bir_lowering=False, debug=False)

    x = nc.dram_tensor("x", [N, C], f32r, kind="ExternalInput")
    qkv_w = nc.dram_tensor("qkv_w", [3 * C, C], f32r, kind="ExternalInput")
    proj_w = nc.dram_tensor("proj_w", [C, C], f32r, kind="ExternalInput")
    proj_b = nc.dram_tensor("proj_b", [C], f32, kind="ExternalInput")
    out = nc.dram_tensor("out", [N, C], f32, kind="ExternalOutput")

    x_r = x.rearrange("(nb p) c -> nb p c", p=P)       # [8, 128, 768]
    w_r = qkv_w.rearrange("(ob p) c -> ob p c", p=P)   # [18, 128, 768]
    pw_r = proj_w.rearrange("(ob p) c -> ob p c", p=P)  # [6, 128, 768]
    out_r = out.rearrange("(nb p) c -> nb p c", p=P)

    with tile.TileContext(nc) as tc:
        with tc.tile_pool(name="persist", bufs=1) as pA:
            # ---- long-lived tensors
            vext = pA.tile([P, NB, H, D + 1], f32r)    # v natural + ones col
            ident_f = pA.tile([P, P], f32)
            ident = pA.tile([P, P], f32r)
            attnT = [pA.tile([P, N], f32r, name=f"attnT{i}") for i in range(CB)]
            pwT = pA.tile([P, CB, C], f32r)            # proj_w^T [c, cb, o2]
            # x^T in two i-halves for finer-grained dependencies
            xTh = [pA.tile([P, CB, 512], f32r, name=f"xTh{i}") for i in range(2)]
            bias_bc = pA.tile([P, C], f32)             # proj_b broadcast

            ones_f = pA.tile([P, NB * H], f32)
            make_identity(nc, ident_f[:])
            nc.vector.tensor_copy(ident[:], ident_f[:])
            nc.vector.memset(ones_f[:], 1.0)
            nc.vector.tensor_copy(
                vext[:, :, :, D:D + 1],
                ones_f[:].rearrange("p (nb h) -> p nb h", nb=NB)[:, :, :, None])
            nc.gpsimd.dma_start(bias_bc[:], proj_b[None, :].to_broadcast((P, C)))

            # W-qk natural blocks: pool spans prelude+merged so the first
            # pair's weights prefetch at the very top.
            p_wqk = ctx_wqk = tc.alloc_tile_pool(name="wqkroll", bufs=4)
            wnat_pre = {}
            for ob in (0, CB):
                t = p_wqk.tile([P, C], f32r, tag="wqknat")
                nc.sync.dma_start(t[:], w_r[ob])
                wnat_pre[ob] = t

            # ============ prelude: transposes + v ============
            with tc.tile_pool(name="pre_sb", bufs=1) as p_pre, \
                 tc.tile_pool(name="pre_roll", bufs=4) as p_roll:
                wTv = p_pre.tile([P, CB, C], f32r)     # v-part of qkv_w^T

                # ---- x -> x^T (48 PE transposes, 4 per psum tile)
                with tc.tile_pool(name="tpsx", bufs=6, space="PSUM") as tpsx, \
                     nc.named_scope("x_transpose"):
                    for nbg in range(2):
                        xnat = []
                        for j in range(4):
                            t = p_roll.tile([P, C], f32r, tag="xnat")
                            nc.sync.dma_start(t[:], x_r[nbg * 4 + j])
                            xnat.append(t)
                        for cb in range(CB):
                            pst = tpsx.tile([P, 512], f32r, tag="tpsx")
                            for j in range(4):
                                nc.tensor.transpose(
                                    pst[:, j * P:(j + 1) * P],
                                    xnat[j][:, cb * P:(cb + 1) * P],
                                    ident[:])
                            nc.any.tensor_copy(xTh[nbg][:, cb, :], pst[:])

                with tc.tile_pool(name="tpsw", bufs=2, space="PSUM") as tpsw:
                    def transpose_w_block(src_row, dst, obi):
                        """transpose one [128, 768] W row-block into dst[:, :, obi*128...]"""
                        wnat = p_roll.tile([P, C], f32r, tag="wnat")
                        nc.sync.dma_start(wnat[:], src_row)
                        pst = tpsw.tile([P, C], f32r, tag="tpsw")
                        for cb in range(CB):
                            nc.tensor.transpose(
                                pst[:, cb * P:(cb + 1) * P],
                                wnat[:, cb * P:(cb + 1) * P], ident[:])
                        nc.any.tensor_copy(
                            dst[:, :, obi * P:(obi + 1) * P],
                            pst[:].rearrange("p (cb k) -> p cb k", cb=CB))

                    # v-part of W^T first (1b depends on it)
                    with nc.named_scope("wT"):
                        for obv in range(CB):
                            transpose_w_block(w_r[2 * CB + obv], wTv, obv)

                    # ---- v (natural layout) into vext
                    with nc.named_scope("v"):
                        for jb in range(NB):
                            ps2 = tpsw.tile([P, C], f32, tag="ps2")
                            for (o0, w) in ((0, 512), (512, 256)):
                                for cb in range(CB):
                                    nc.tensor.matmul(
                                        ps2[:, o0:o0 + w],
                                        xTh[jb // 4][:, cb,
                                                     (jb % 4) * P:(jb % 4 + 1) * P],
                                        wTv[:, cb, o0:o0 + w],
                                        start=(cb == 0), stop=(cb == CB - 1))
                            nc.any.tensor_copy(
                                vext[:, jb, :, 0:D],
                                ps2[:].rearrange("p (h d) -> p h d", h=H))

                    # pw^T last: only needed by proj at the very end
                    with nc.named_scope("pwT"):
                        for obp in range(CB):
                            transpose_w_block(pw_r[obp], pwT, obp)

            # ============ interleaved qk + attention, per head pair ============
            with tc.tile_pool(name="qkroll", bufs=3) as p_qk, \
                 tc.tile_pool(name="etpool", bufs=4) as p_et, \
                 tc.tile_pool(name="ph2sm", bufs=1) as p_sm, \
                 tc.tile_pool(name="ph2dram", bufs=2, space="DRAM") as p_dram, \
                 tc.tile_pool(name="mm1", bufs=2, space="PSUM") as mm1p, \
                 tc.tile_pool(name="pss", bufs=2, space="PSUM") as pssp, \
                 tc.tile_pool(name="pso", bufs=1, space="PSUM") as psop, \
                 nc.named_scope("attention"):
                for hb in range(CB):
                    # ---- produce q^T (ob=hb) and k^T (ob=6+hb) for this pair
                    qk_t = {}
                    for ob in (hb, CB + hb):
                        if ob in wnat_pre:
                            wnat = wnat_pre.pop(ob)
                        else:
                            wnat = p_wqk.tile([P, C], f32r, tag="wqknat")
                            nc.sync.dma_start(wnat[:], w_r[ob])
                        wtq = p_wqk.tile([P, CB, P], f32r, tag="wqk")
                        psa = mm1p.tile([P, 512], f32r, tag="mm1")
                        for cb in range(4):
                            nc.tensor.transpose(
                                psa[:, cb * P:(cb + 1) * P],
                                wnat[:, cb * P:(cb + 1) * P], ident[:])
                        nc.vector.tensor_copy(
                            wtq[:, 0:4, :],
                            psa[:].rearrange("p (cb k) -> p cb k", cb=4))
                        psb = mm1p.tile([P, 512], f32r, tag="mm1")
                        for cb in range(2):
                            nc.tensor.transpose(
                                psb[:, cb * P:(cb + 1) * P],
                                wnat[:, (4 + cb) * P:(5 + cb) * P], ident[:])
                        nc.vector.tensor_copy(
                            wtq[:, 4:6, :],
                            psb[:, 0:256].rearrange("p (cb k) -> p cb k", cb=2))
                        t = p_qk.tile([P, N], f32r, tag="qkt")
                        qk_t[ob] = t
                        for ic in range(2):
                            ps1 = mm1p.tile([P, 512], f32, tag="mm1")
                            for cb in range(CB):
                                nc.tensor.matmul(
                                    ps1[:], wtq[:, cb, :],
                                    xTh[ic][:, cb, :],
                                    start=(cb == 0), stop=(cb == CB - 1))
                            nc.vector.tensor_copy(
                                t[:, ic * 512:(ic + 1) * 512], ps1[:])
                    qt, kt = qk_t[hb], qk_t[CB + hb]

                    for h in (2 * hb, 2 * hb + 1):
                        hp = h % 2
                        r0, r1 = hp * D, hp * D + D
                        # S^T = k^T.T @ q^T ; E^T = exp(S^T/8)
                        ets = []
                        for jbg in range(4):
                            et = p_et.tile([P, 2, N], f32r, tag="et")
                            ets.append(et)
                            for jj in range(2):
                                jb = jbg * 2 + jj
                                ps_s = pssp.tile([P, N], f32, tag="pss")
                                for ic in range(2):
                                    nc.tensor.matmul(
                                        ps_s[:, ic * 512:(ic + 1) * 512],
                                        kt[r0:r1, jb * P:(jb + 1) * P],
                                        qt[r0:r1, ic * 512:(ic + 1) * 512],
                                        start=True, stop=True)
                                nc.scalar.activation(
                                    et[:, jj, :], ps_s[:],
                                    mybir.ActivationFunctionType.Exp, scale=SCALE)
                        # O'^T = [v|1].T @ E^T (rows 0..63 out, row 64 rowsum)
                        ps_o = psop.tile([D + 1, N], f32, tag="pso")
                        for jb in range(NB):
                            for ic in range(2):
                                nc.tensor.matmul(
                                    ps_o[:, ic * 512:(ic + 1) * 512],
                                    vext[:, jb, h, :],
                                    ets[jb // 2][:, jb % 2, ic * 512:(ic + 1) * 512],
                                    start=(jb == 0), stop=(jb == NB - 1))
                        # evacuate PSUM promptly; normalize in place afterwards.
                        r = p_sm.tile([1, N], f32, tag="r", bufs=2)
                        rs = p_sm.tile([1, N], f32, tag="rs")
                        scr = p_sm.tile([1, N], f32, tag="scr")
                        nc.scalar.copy(rs[:], ps_o[D:D + 1, :])
                        nc.vector.tensor_copy(attnT[hb][r0:r1, :], ps_o[0:D, :])
                        nc.vector.reciprocal_approx_accurate(r[:], rs[:], scr[:])
                        rb = p_sm.tile([P, N], f32, tag="rb", bufs=1)
                        rdram = p_dram.tile([1, N], f32, tag="rdram")
                        nc.sync.dma_start(rdram[:], r[:])
                        nc.gpsimd.dma_start(
                            rb[:], rdram[0, :][None, :].to_broadcast((P, N)))
                        nc.vector.tensor_tensor(
                            attnT[hb][r0:r1, :],
                            attnT[hb][r0:r1, :].bitcast(f32),
                            rb[r0:r1, :], mybir.AluOpType.mult)

                # ---- proj, two passes: cb 0..4 gap-fill during the last
                # pair's attention; the cb=5 contribution lands after the
                # final heads normalize.
                with nc.named_scope("proj"):
                    osbs = []
                    for nb in range(NB):
                        osb = p_sm.tile([P, C], f32, tag="osb", bufs=8)
                        osbs.append(osb)
                        for (o0, w) in ((0, 512), (512, 256)):
                            ps3 = mm1p.tile([P, 512], f32, tag="mm1")
                            for cb in range(CB - 1):
                                nc.tensor.matmul(
                                    ps3[:, 0:w],
                                    attnT[cb][:, nb * P:(nb + 1) * P],
                                    pwT[:, cb, o0:o0 + w],
                                    start=(cb == 0), stop=(cb == CB - 2))
                            nc.vector.tensor_tensor(
                                osb[:, o0:o0 + w], ps3[:, 0:w],
                                bias_bc[:, o0:o0 + w], mybir.AluOpType.add)
                    for nb in range(NB):
                        osb = osbs[nb]
                        for (o0, w) in ((0, 512), (512, 256)):
                            ps3 = mm1p.tile([P, 512], f32, tag="mm1")
                            nc.tensor.matmul(
                                ps3[:, 0:w],
                                attnT[CB - 1][:, nb * P:(nb + 1) * P],
                                pwT[:, CB - 1, o0:o0 + w],
                                start=True, stop=True)
                            nc.vector.tensor_tensor(
                                osb[:, o0:o0 + w],
                                osb[:, o0:o0 + w],
                                ps3[:, 0:w], mybir.AluOpType.add)
                        nc.sync.dma_start(out_r[nb], osb[:])

            p_wqk.release()


    nc.finalize()
    return nc


_NC_CACHE = None


def kernel(x, qkv_w, proj_w, proj_b):
    """Full inputs -> full output. x: [8, 1024, 768]."""
    global _NC_CACHE
    from concourse.bass_utils import run_bass_kernel_spmd

    if _NC_CACHE is None:
        _NC_CACHE = build_attention_bass()
    nc = _NC_CACHE

    x = np.ascontiguousarray(np.asarray(x, dtype=np.float32))
    qkv_w = np.ascontiguousarray(np.asarray(qkv_w, dtype=np.float32))
    proj_w = np.ascontiguousarray(np.asarray(proj_w, dtype=np.float32))
    proj_b = np.ascontiguousarray(np.asarray(proj_b, dtype=np.float32))

    in_maps = [
        {"x": x[b], "qkv_w": qkv_w, "proj_w": proj_w, "proj_b": proj_b}
        for b in range(B)
    ]
    res = run_bass_kernel_spmd(nc, in_maps, core_ids=list(range(B)))
    return np.stack([res.results[b]["out"] for b in range(B)], axis=0)
